# revision 1
# baseline (speedup 1.0000x reference)
"""CongestionGCN on 8 Trainium2 NeuronCores.

Graph/data-parallel sharding: nodes split contiguously across 8 cores
(12500 each, padded to 12544 = 98*128). Edges partitioned by dst node.
Message passing runs as: indirect-DMA gather of src rows from a
replicated node-major h table, then a one-hot matmul on the tensor
engine that performs the scatter-add and the mean (1/deg) scaling in
one pass, accumulating msg^T (feature-major) in PSUM. Dense GEMMs,
BatchNorm (+ cross-core AllReduce of batch stats), ReLU and residual
run feature-major out of SBUF-resident buffers. Each layer's output
shard is transposed back to node-major and AllGathered to rebuild the
replicated h for the next layer's gathers.

conv_b is not applied on device: BatchNorm subtracts the batch mean, so
a per-feature bias added before BN cancels exactly.
"""

import numpy as np

N = 100000
E = 600000
IN = 12
H = 128
OUT = 2
L = 3
EPS = 1e-5

NCORES = 8
P = 128
NPC = N // NCORES            # 12500 real nodes per core
NT = (NPC + P - 1) // P      # 98 dst tiles per core
NPC_PAD = NT * P             # 12544
NFULL = NCORES * NPC_PAD     # 100352 rows in the replicated padded table
GW = 512                     # free-dim group width for dense GEMM / BN passes

_cache = {}


NW = 4                       # table windows (2 core shards each, 25088 rows)
WROWS = 2 * NPC_PAD          # 25088 <= int16 range
NG = (NT + 3) // 4           # 25 groups of up to 4 tiles (512 dsts)


def _host_prep(features, edge_index, **kw):
    """Edge partitioning + padding for windowed dma_gather.

    Edges sorted by (core, group, window, tile, src). Each (tile, window)
    segment is padded to C_tw*128 slots (pad -> window row 0, weight 0), so
    one dma_gather per (group, window) fetches ntiles*C_tw chunks whose
    chunk boundaries align with dst tiles.
    Returns idx16 [8, NG, P, CAP], meta [8, NG, P, 2*4*NW*C_tw], featT, C_tw.
    """
    src = np.asarray(edge_index[0], dtype=np.int64)
    dst = np.asarray(edge_index[1], dtype=np.int64)

    deg = np.bincount(dst, minlength=N).astype(np.float32)
    inv_deg = (1.0 / np.maximum(deg, 1.0)).astype(np.float32)

    core = dst // NPC
    ltile = (dst % NPC) // P                 # 0..97
    group = ltile // 4                       # 0..24
    tloc = ltile % 4
    dst_rel = (dst % NPC) % P
    gsrc = (src // NPC) * NPC_PAD + (src % NPC)
    win = gsrc // WROWS
    idx_rel = (gsrc - win * WROWS).astype(np.int16)

    cell = ((core * NG + group) * NW + win) * 4 + tloc   # (c,g,w,t)
    ncell = NCORES * NG * NW * 4
    order = np.lexsort((src, cell))
    cell_s = cell[order]
    counts = np.bincount(cell_s, minlength=ncell)
    C_tw = int((counts.max() + P - 1) // P)
    CAPS = C_tw * P                          # slots per (tile, window) segment
    CAP = 4 * CAPS                           # idx16 cols per (group, window): 4*CAPS/16*... actually slots
    starts = np.zeros(ncell, dtype=np.int64)
    starts[1:] = np.cumsum(counts)[:-1]
    pos = np.arange(E, dtype=np.int64) - starts[cell_s]

    idx_s = idx_rel[order]
    rel_s = dst_rel[order].astype(np.float32)
    w_s = inv_deg[dst[order]]
    core_s = core[order]
    grp_s = group[order]
    win_s = win[order]
    tloc_s = tloc[order]

    # gather slot within the (core, group, window) sequence
    slot = tloc_s * CAPS + pos
    # idx16 layout: index k of a gather lives at [k % 16, k // 16],
    # replicated across the 8 groups of 16 partitions; window blocks are
    # packed contiguously: ntg(g)*CAPS/16 cols each.
    idxcols_per_w = np.where(grp_s == NG - 1, 2 * CAPS // 16, 4 * CAPS // 16)
    colbase = win_s * idxcols_per_w
    IDXCOLS = 4 * (4 * CAPS // 16)
    idx16 = np.zeros((NCORES, NG, 16, IDXCOLS), dtype=np.int16)
    idx16[core_s, grp_s, slot % 16, colbase + slot // 16] = idx_s

    # meta chunk id on device: ((t*NW + w)*C_tw + cc)
    cc = pos // P
    lane = pos % P
    m = (tloc_s * NW + win_s) * C_tw + cc
    CPG = 4 * NW * C_tw
    meta = np.zeros((NCORES, NG, P, 2 * CPG), dtype=np.float32)
    meta[core_s, grp_s, lane, 2 * m] = rel_s
    meta[core_s, grp_s, lane, 2 * m + 1] = w_s

    idx16_full = np.broadcast_to(
        idx16[:, :, None, :, :], (NCORES, NG, 8, 16, IDXCOLS)
    ).reshape(NCORES, NG, P, IDXCOLS)

    feats = np.asarray(features, dtype=np.float32)
    featT = np.zeros((NCORES, IN, NPC_PAD), dtype=np.float32)
    for c in range(NCORES):
        featT[c, :, :NPC] = feats[c * NPC:(c + 1) * NPC].T

    return np.ascontiguousarray(idx16_full), meta, featT, C_tw


def _build_program(C, nlayers=L, use_ar=True, use_ag=True, gather_mode='full', shared=True, use_tr=True, use_bn=True, bnmode=2, gdt='bf16'):  # C == C_tw
    import concourse.bacc as bacc
    import concourse.tile as tile
    from concourse import bass, mybir
    from concourse.masks import make_identity

    f32 = mybir.dt.float32
    i32 = mybir.dt.int32
    gdtype = mybir.dt.bfloat16 if gdt == "bf16" else f32

    nc = bacc.Bacc("TRN2", target_bir_lowering=False)

    C_tw = C
    CAPS = C_tw * P
    IDXCOLS = 4 * (4 * CAPS // 16)
    CPG = 4 * NW * C_tw
    i16 = mybir.dt.int16
    featT_p = nc.declare_dram_parameter("featT", [IN, NPC_PAD], f32, isOutput=False)
    idx_p = nc.declare_dram_parameter("idx", [NG, P, IDXCOLS], i16, isOutput=False)
    meta_p = nc.declare_dram_parameter("meta", [NG, P, 2 * CPG], f32, isOutput=False)
    embW_p = nc.declare_dram_parameter("embW", [IN, H], f32, isOutput=False)
    embb_p = nc.declare_dram_parameter("embb", [H, 1], f32, isOutput=False)
    Wself_p = nc.declare_dram_parameter("Wself", [L, H, H], f32, isOutput=False)
    Wneigh_p = nc.declare_dram_parameter("Wneigh", [L, H, H], f32, isOutput=False)
    bng_p = nc.declare_dram_parameter("bng", [L, H, 1], f32, isOutput=False)
    bnb_p = nc.declare_dram_parameter("bnb", [L, H, 1], f32, isOutput=False)
    W1_p = nc.declare_dram_parameter("W1", [H, H // 2], f32, isOutput=False)
    b1_p = nc.declare_dram_parameter("b1", [H // 2, 1], f32, isOutput=False)
    W2_p = nc.declare_dram_parameter("W2", [H // 2, OUT], f32, isOutput=False)
    b2_p = nc.declare_dram_parameter("b2", [OUT, 1], f32, isOutput=False)
    out_p = nc.declare_dram_parameter("out", [OUT, NPC_PAD], f32, isOutput=True)

    groups = [(s, min(GW, NPC_PAD - s)) for s in range(0, NPC_PAD, GW)]
    rg = [list(range(NCORES))]

    with tile.TileContext(nc) as tc:
        with (
            tc.tile_pool(name="persist", bufs=1) as pp,
            tc.tile_pool(name="pdram", bufs=1, space="DRAM") as pd,
            tc.tile_pool(name="gpool", bufs=8) as gpool,
            tc.tile_pool(name="ohpool", bufs=8) as ohpool,
            tc.tile_pool(name="mpool", bufs=3) as mpool,
            tc.tile_pool(name="grp", bufs=2) as grp,
            tc.tile_pool(name="small", bufs=8) as small,
            tc.tile_pool(name="trp", bufs=4) as trp,
            tc.tile_pool(name="mps", bufs=4, space="PSUM") as mps,
            tc.tile_pool(name="xps", bufs=2, space="PSUM") as xps,
            tc.tile_pool(name="tps", bufs=2, space="PSUM") as tps,
        ):
            # --- persistent SBUF state ---
            bufA = pp.tile([P, NPC_PAD], f32, tag="bufA", name="bufA")
            bufB = pp.tile([P, NPC_PAD], f32, tag="bufB", name="bufB")
            iota_t = pp.tile([P, P], f32, tag="iota_t", name="iota_t")
            nc.gpsimd.iota(iota_t[:], pattern=[[1, P]], base=0,
                           channel_multiplier=0,
                           allow_small_or_imprecise_dtypes=True)
            ident = pp.tile([P, P], f32, tag="ident", name="ident")
            make_identity(nc, ident[:])
            eps_t = pp.tile([P, 1], f32, tag="eps_t", name="eps_t")
            nc.gpsimd.memset(eps_t[:], EPS)

            embW_t = pp.tile([IN, H], f32, tag="embW_t", name="embW_t")
            nc.sync.dma_start(embW_t[:], embW_p[:])
            embb_t = pp.tile([H, 1], f32, tag="embb_t", name="embb_t")
            nc.sync.dma_start(embb_t[:], embb_p[:])
            W1_t = pp.tile([H, H // 2], f32, tag="W1_t", name="W1_t")
            nc.sync.dma_start(W1_t[:], W1_p[:])
            b1_t = pp.tile([H // 2, 1], f32, tag="b1_t", name="b1_t")
            nc.sync.dma_start(b1_t[:], b1_p[:])
            W2_t = pp.tile([H // 2, OUT], f32, tag="W2_t", name="W2_t")
            nc.sync.dma_start(W2_t[:], W2_p[:])
            b2_t = pp.tile([OUT, 1], f32, tag="b2_t", name="b2_t")
            nc.sync.dma_start(b2_t[:], b2_p[:])

            # --- internal DRAM ---
            h_node = [
                pd.tile([NFULL, H], gdtype,
                        addr_space="Shared" if shared else "Local",
                        tag=f"h_node{l}", name=f"h_node{l}")
                for l in range(L)
            ]
            ag_in = [
                pd.tile([NPC_PAD, H], gdtype, tag=f"ag_in{l}", name=f"ag_in{l}")
                for l in range(L)
            ]
            ar_in = [
                pd.tile([P, 2], f32, tag=f"ar_in{l}", name=f"ar_in{l}")
                for l in range(L)
            ]
            ar_out = [
                pd.tile([P, 2], f32,
                        addr_space="Shared" if shared else "Local",
                        tag=f"ar_out{l}", name=f"ar_out{l}")
                for l in range(L)
            ]

            def transpose_to(l_next):
                """bufA (feature-major) -> node-major shard -> ag_in[l_next],
                then AllGather into h_node[l_next]. Transposes are staged in
                [128, 512] tiles so each 4-tile group is one DMA."""
                for (s, w) in groups:
                    ntg = (w + P - 1) // P
                    stage = trp.tile([P, GW], gdtype, tag="tr")
                    for k in range(ntg):
                        ps = tps.tile([P, P], f32, tag="tps")
                        nc.tensor.transpose(
                            out=ps[:], in_=bufA[:, s + k * P:s + (k + 1) * P],
                            identity=ident[:],
                        )
                        nc.scalar.copy(out=stage[:, k * P:(k + 1) * P], in_=ps[:])
                    nc.scalar.dma_start(
                        ag_in[l_next][s:s + w, :].rearrange(
                            "(b p) f -> p b f", p=P),
                        stage[:, :w].rearrange("p (b f) -> p b f", f=P),
                    )
                if use_ag:
                    nc.gpsimd.collective_compute(
                        "AllGather",
                        mybir.AluOpType.bypass,
                        ins=[ag_in[l_next].opt()],
                        outs=[h_node[l_next].opt()],
                        replica_groups=rg,
                    )
                else:
                    nc.sync.dma_start(
                        h_node[l_next][:NPC_PAD, :], ag_in[l_next][:]
                    )

            # ---------------- embedding ----------------
            for (s, w) in groups:
                ft = mpool.tile([IN, GW], f32, tag="ft")
                nc.sync.dma_start(ft[:, :w], featT_p[:, s:s + w])
                ps = xps.tile([P, GW], f32, tag="xps")
                nc.tensor.matmul(out=ps[:, :w], lhsT=embW_t[:], rhs=ft[:, :w],
                                 start=True, stop=True)
                nc.vector.tensor_scalar_add(bufA[:, s:s + w], ps[:, :w], embb_t[:])
            nc.gpsimd.memset(bufA[:, NPC:], 0.0)
            if nlayers > 0 and use_tr:
                transpose_to(0)

            # ---------------- conv layers ----------------
            for l in range(nlayers):
                Wself_t = small.tile([H, H], f32, tag="ws", bufs=2)
                nc.sync.dma_start(Wself_t[:], Wself_p[l])
                Wneigh_t = small.tile([H, H], f32, tag="wn", bufs=2)
                nc.sync.dma_start(Wneigh_t[:], Wneigh_p[l])
                bng_t = small.tile([H, 1], f32, tag="bng", bufs=2)
                nc.sync.dma_start(bng_t[:], bng_p[l])
                bnb_t = small.tile([H, 1], f32, tag="bnb", bufs=2)
                nc.sync.dma_start(bnb_t[:], bnb_p[l])

                s1p = small.tile([P, 32], f32, tag="s1p", bufs=2)
                s2p = small.tile([P, 32], f32, tag="s2p", bufs=2)

                # ---- phase 1: messages + GEMM + stats ----
                for gi, (s, w) in enumerate(groups):
                    msg_g = grp.tile([P, GW], f32, tag="msg")
                    ntg = (w + P - 1) // P
                    capw = ntg * CAPS          # idxs per (group, window) gather
                    idxc_w = capw // 16
                    if gather_mode == "none":
                        nc.gpsimd.memset(msg_g[:, :w], 0.0)
                    else:
                        it = mpool.tile([P, IDXCOLS], i16, tag="idx")
                        nc.sync.dma_start(it[:, :4 * idxc_w],
                                          idx_p[gi][:, :4 * idxc_w])
                        mt = mpool.tile([P, 2 * CPG], f32, tag="meta")
                        nc.sync.dma_start(mt[:], meta_p[gi])
                        Gs = []
                        for wi in range(NW):
                            gw_t = gpool.tile([P, 4 * CAPS // P * H], gdtype, tag="g")
                            nc.gpsimd.dma_gather(
                                out_ap=gw_t[:, :capw // P * H].rearrange(
                                    "p (c e) -> p c e", e=H),
                                in_ap=h_node[l][wi * WROWS:(wi + 1) * WROWS, :],
                                idxs_ap=it[:, wi * idxc_w:(wi + 1) * idxc_w],
                                num_idxs=capw,
                                num_idxs_reg=capw,
                                elem_size=H,
                            )
                            Gs.append(gw_t)
                        msg_ps = mps.tile([P, GW], f32, tag="mps")
                        for t in range(ntg):
                            nmm = NW * C_tw
                            i_mm = 0
                            for wi in range(NW):
                                for cc2 in range(C_tw):
                                    ci = t * C_tw + cc2
                                    mcol = (t * NW + wi) * C_tw + cc2
                                    oh = ohpool.tile([P, P], gdtype, tag="oh")
                                    nc.vector.tensor_scalar(
                                        out=oh[:],
                                        in0=iota_t[:],
                                        scalar1=mt[:, 2 * mcol:2 * mcol + 1],
                                        scalar2=mt[:, 2 * mcol + 1:2 * mcol + 2],
                                        op0=mybir.AluOpType.is_equal,
                                        op1=mybir.AluOpType.mult,
                                    )
                                    nc.tensor.matmul(
                                        out=msg_ps[:, t * P:(t + 1) * P],
                                        lhsT=Gs[wi][:, ci * H:(ci + 1) * H],
                                        rhs=oh[:],
                                        start=(i_mm == 0), stop=(i_mm == nmm - 1),
                                    )
                                    i_mm += 1
                        nc.vector.tensor_copy(out=msg_g[:, :w], in_=msg_ps[:, :w])

                    ps = xps.tile([P, GW], f32, tag="xps")
                    nc.tensor.matmul(out=ps[:, :w], lhsT=Wself_t[:],
                                     rhs=bufA[:, s:s + w], start=True, stop=False)
                    nc.tensor.matmul(out=ps[:, :w], lhsT=Wneigh_t[:],
                                     rhs=msg_g[:, :w], start=False, stop=True)
                    nc.vector.tensor_copy(out=bufB[:, s:s + w], in_=ps[:, :w])

                    wr = w if s + w <= NPC else max(0, NPC - s)
                    if (use_bn or bnmode >= 1) and wr > 0:
                        nc.vector.reduce_sum(
                            s1p[:, gi:gi + 1], bufB[:, s:s + wr],
                            axis=mybir.AxisListType.X,
                        )
                        sq = grp.tile([P, GW], f32, tag="sq")
                        nc.scalar.activation(
                            sq[:, :wr], bufB[:, s:s + wr],
                            mybir.ActivationFunctionType.Square,
                            accum_out=s2p[:, gi:gi + 1],
                        )

                # ---- BN stats all-reduce ----
                if not use_bn or bnmode == 1:
                    for (s, w) in groups:
                        if l == 0:
                            nc.vector.tensor_copy(out=bufA[:, s:s + w],
                                                  in_=bufB[:, s:s + w])
                        else:
                            nc.vector.tensor_tensor(
                                out=bufA[:, s:s + w], in0=bufB[:, s:s + w],
                                in1=bufA[:, s:s + w], op=mybir.AluOpType.add)
                    nc.gpsimd.memset(bufA[:, NPC:], 0.0)
                    if l + 1 < nlayers and use_tr:
                        transpose_to(l + 1)
                    continue
                st = small.tile([P, 2], f32, tag="st", bufs=2)
                nc.vector.reduce_sum(st[:, 0:1], s1p[:, :len(groups)],
                                     axis=mybir.AxisListType.X)
                nc.vector.reduce_sum(st[:, 1:2], s2p[:, :len(groups)],
                                     axis=mybir.AxisListType.X)
                if use_ar:
                    nc.sync.dma_start(ar_in[l][:], st[:])
                    nc.gpsimd.collective_compute(
                        "AllReduce",
                        mybir.AluOpType.add,
                        ins=[ar_in[l].opt()],
                        outs=[ar_out[l].opt()],
                        replica_groups=rg,
                    )
                    sg = small.tile([P, 2], f32, tag="sg", bufs=2)
                    nc.sync.dma_start(sg[:], ar_out[l][:])
                else:
                    sg = st

                mu = small.tile([P, 1], f32, tag="mu", bufs=2)
                nc.vector.tensor_scalar_mul(mu[:], sg[:, 0:1], 1.0 / N)
                ex2 = small.tile([P, 1], f32, tag="ex2", bufs=2)
                nc.vector.tensor_scalar_mul(ex2[:], sg[:, 1:2], 1.0 / N)
                var = small.tile([P, 1], f32, tag="var", bufs=2)
                nc.vector.tensor_tensor(out=var[:], in0=mu[:], in1=mu[:],
                                        op=mybir.AluOpType.mult)
                nc.vector.tensor_tensor(out=var[:], in0=ex2[:], in1=var[:],
                                        op=mybir.AluOpType.subtract)
                sd = small.tile([P, 1], f32, tag="sd", bufs=2)
                nc.scalar.activation(sd[:], var[:],
                                     mybir.ActivationFunctionType.Sqrt,
                                     bias=eps_t[:])
                rstd = small.tile([P, 1], f32, tag="rstd", bufs=2)
                nc.vector.reciprocal(rstd[:], sd[:])
                a_t = small.tile([P, 1], f32, tag="a_t", bufs=2)
                nc.vector.tensor_tensor(out=a_t[:], in0=bng_t[:], in1=rstd[:],
                                        op=mybir.AluOpType.mult)
                b_t = small.tile([P, 1], f32, tag="b_t", bufs=2)
                nc.vector.tensor_tensor(out=b_t[:], in0=mu[:], in1=a_t[:],
                                        op=mybir.AluOpType.mult)
                nc.vector.tensor_tensor(out=b_t[:], in0=bnb_t[:], in1=b_t[:],
                                        op=mybir.AluOpType.subtract)

                # ---- phase 2: BN + ReLU (+ residual) ----
                if bnmode == 3:
                    nc.sync.dma_start(ar_in[l][:, 0:1], a_t[:])
                    nc.sync.dma_start(ar_in[l][:, 1:2], b_t[:])
                for (s, w) in groups:
                    if bnmode == 3:
                        nc.scalar.activation(
                            bufA[:, s:s + w], bufB[:, s:s + w],
                            mybir.ActivationFunctionType.Relu,
                        )
                        continue
                    y = grp.tile([P, GW], f32, tag="y")
                    nc.vector.tensor_scalar(
                        out=y[:, :w], in0=bufB[:, s:s + w],
                        scalar1=a_t[:], scalar2=b_t[:],
                        op0=mybir.AluOpType.mult, op1=mybir.AluOpType.add,
                    )
                    if l == 0:
                        nc.scalar.activation(
                            bufA[:, s:s + w], y[:, :w],
                            mybir.ActivationFunctionType.Relu,
                        )
                    else:
                        y2 = grp.tile([P, GW], f32, tag="y2")
                        nc.scalar.activation(
                            y2[:, :w], y[:, :w],
                            mybir.ActivationFunctionType.Relu,
                        )
                        nc.vector.tensor_tensor(
                            out=bufA[:, s:s + w], in0=y2[:, :w],
                            in1=bufA[:, s:s + w], op=mybir.AluOpType.add,
                        )
                nc.gpsimd.memset(bufA[:, NPC:], 0.0)

                if l + 1 < nlayers and use_tr:
                    transpose_to(l + 1)

            # ---------------- MLP head ----------------
            for (s, w) in groups:
                ps1 = xps.tile([H // 2, GW], f32, tag="xps")
                nc.tensor.matmul(out=ps1[:, :w], lhsT=W1_t[:],
                                 rhs=bufA[:, s:s + w], start=True, stop=True)
                z1 = grp.tile([H // 2, GW], f32, tag="z1")
                nc.scalar.activation(
                    z1[:, :w], ps1[:, :w],
                    mybir.ActivationFunctionType.Relu, bias=b1_t[:],
                )
                ps2 = tps.tile([OUT, GW], f32, tag="tps")
                nc.tensor.matmul(out=ps2[:, :w], lhsT=W2_t[:],
                                 rhs=z1[:, :w], start=True, stop=True)
                o = trp.tile([OUT, GW], f32, tag="tr")
                nc.vector.tensor_scalar_add(o[:, :w], ps2[:, :w], b2_t[:])
                nc.sync.dma_start(out_p[:, s:s + w], o[:, :w])

    nc.compile()
    return nc


def kernel(**inputs):
    from concourse.bass_utils import run_bass_kernel_spmd

    idx, meta, featT, C = _host_prep(**inputs)

    key = ("prog", C)
    if key not in _cache:
        _cache[key] = _build_program(C)
    nc = _cache[key]

    f32 = np.float32
    embW = np.ascontiguousarray(np.asarray(inputs["emb_W"], f32))
    embb = np.asarray(inputs["emb_b"], f32).reshape(H, 1)
    Wself = np.ascontiguousarray(np.asarray(inputs["Wself"], f32))
    Wneigh = np.ascontiguousarray(np.asarray(inputs["Wneigh"], f32))
    bng = np.asarray(inputs["bn_gamma"], f32).reshape(L, H, 1)
    bnb = np.asarray(inputs["bn_beta"], f32).reshape(L, H, 1)
    W1 = np.ascontiguousarray(np.asarray(inputs["W1"], f32))
    b1 = np.asarray(inputs["b1"], f32).reshape(H // 2, 1)
    W2 = np.ascontiguousarray(np.asarray(inputs["W2"], f32))
    b2 = np.asarray(inputs["b2"], f32).reshape(OUT, 1)

    in_maps = []
    for c in range(NCORES):
        in_maps.append({
            "featT": np.ascontiguousarray(featT[c]),
            "idx": np.ascontiguousarray(idx[c]),
            "meta": np.ascontiguousarray(meta[c]),
            "embW": embW, "embb": embb,
            "Wself": Wself, "Wneigh": Wneigh,
            "bng": bng, "bnb": bnb,
            "W1": W1, "b1": b1, "W2": W2, "b2": b2,
        })

    global _last_in_maps
    _last_in_maps = in_maps

    res = run_bass_kernel_spmd(nc, in_maps, list(range(NCORES))).results
    out = np.concatenate(
        [res[c]["out"][:, :NPC].T for c in range(NCORES)], axis=0
    )
    return out.astype(np.float32)


if __name__ == "__main__":
    pass



# revision 2
# speedup vs baseline: 2.1920x; 2.1920x over previous
"""CongestionGCN on 8 Trainium2 NeuronCores.

Graph/data-parallel sharding: nodes split contiguously across 8 cores
(12500 each, padded to 12544 = 98*128). Edges partitioned by dst node.

v2 layout:
- Layer 1's message aggregation is linear in the raw inputs, so the
  host folds it away: msg1 = (S_w X) @ embW + mask*emb_b, and both the
  Wself and Wneigh contributions collapse to two 13-row GEMMs against
  host-shipped [X^T;1] and [(S_w X)^T;mask]. No gather, no AllGather,
  no embedding GEMM for layer 1.
- Layers 2/3 message passing: indirect-DMA gather of src rows from a
  node-major table (4 windows = src quarters across all cores, int16
  addressable), spread across the 4 SWDGE queues so the Q7 pairs
  generate descriptors in parallel. A one-hot matmul on the tensor
  engine performs scatter-add + mean scaling, accumulating msg^T in
  PSUM.
- Each layer's output is transposed back to node-major and AllGathered
  in 4 quarter chunks, so window-q gathers of the next layer start as
  soon as chunk q lands.
- BatchNorm batch stats cross-core via AllReduce. conv_b is dropped:
  BN subtracts the batch mean, so a pre-BN per-feature bias cancels.
"""

import numpy as np

N = 100000
E = 600000
IN = 12
H = 128
OUT = 2
L = 3
EPS = 1e-5

NCORES = 8
P = 128
NPC = N // NCORES            # 12500 real nodes per core
NT = (NPC + P - 1) // P      # 98 dst tiles per core
NPC_PAD = NT * P             # 12544
GW = 512                     # free-dim group width for dense GEMM / BN passes

NW = 4                       # src windows = shard quarters
QB = [0, 3200, 6400, 9472, 12544]      # quarter boundaries (128-aligned)
QS = [3200, 3200, 3072, 3072]          # quarter sizes
WR = [8 * q for q in QS]               # window table rows (<= int16 range)
QT = [25, 25, 24, 24]                  # tiles per quarter
NG = (NT + 3) // 4           # 25 groups of up to 4 dst tiles (512 dsts)

_cache = {}


def _host_prep(features, edge_index, emb_W, emb_b, Wself, Wneigh, **kw):
    src = np.asarray(edge_index[0], dtype=np.int64)
    dst = np.asarray(edge_index[1], dtype=np.int64)
    X = np.asarray(features, dtype=np.float32)

    deg = np.bincount(dst, minlength=N).astype(np.float32)
    inv_deg = (1.0 / np.maximum(deg, 1.0)).astype(np.float32)

    # ---- layer-1 folding: SwX = diag(inv_deg) * segment_sum(X[src], dst)
    SwX = np.empty((N, IN), np.float32)
    for j in range(IN):
        SwX[:, j] = np.bincount(dst, weights=X[src, j], minlength=N)
    SwX *= inv_deg[:, None]
    mask = (deg > 0).astype(np.float32)

    emb_W = np.asarray(emb_W, np.float32)
    emb_b = np.asarray(emb_b, np.float32)
    Ws0 = np.asarray(Wself[0], np.float32)
    Wn0 = np.asarray(Wneigh[0], np.float32)
    lhs1 = np.vstack([emb_W @ Ws0, emb_b[None, :] @ Ws0])   # [13, H]
    lhs2 = np.vstack([emb_W @ Wn0, emb_b[None, :] @ Wn0])   # [13, H]

    featT = np.zeros((NCORES, IN + 1, NPC_PAD), np.float32)
    hXT = np.zeros((NCORES, IN + 1, NPC_PAD), np.float32)
    for c in range(NCORES):
        sl = slice(c * NPC, (c + 1) * NPC)
        featT[c, :IN, :NPC] = X[sl].T
        featT[c, IN, :NPC] = 1.0
        hXT[c, :IN, :NPC] = SwX[sl].T
        hXT[c, IN, :NPC] = mask[sl]

    # ---- edge partitioning for layers 2/3 gathers
    core = dst // NPC
    ltile = (dst % NPC) // P                 # 0..97
    group = ltile // 4                       # 0..24
    tloc = ltile % 4
    dst_rel = (dst % NPC) % P
    src_c = src // NPC
    src_r = src % NPC
    win = np.searchsorted(QB, src_r, side='right') - 1
    qs = np.asarray(QS, np.int64)
    qb = np.asarray(QB[:4], np.int64)
    idx_rel = (src_c * qs[win] + (src_r - qb[win])).astype(np.int16)

    cell = ((core * NG + group) * NW + win) * 4 + tloc
    ncell = NCORES * NG * NW * 4
    order = np.lexsort((src, cell))
    cell_s = cell[order]
    counts = np.bincount(cell_s, minlength=ncell)
    C_tw = int((counts.max() + P - 1) // P)
    CAPS = C_tw * P
    starts = np.zeros(ncell, dtype=np.int64)
    starts[1:] = np.cumsum(counts)[:-1]
    pos = np.arange(E, dtype=np.int64) - starts[cell_s]

    idx_s = idx_rel[order]
    rel_s = dst_rel[order].astype(np.float32)
    w_s = inv_deg[dst[order]]
    core_s = core[order]
    grp_s = group[order]
    win_s = win[order]
    tloc_s = tloc[order]

    slot = tloc_s * CAPS + pos
    idxcols_per_w = np.where(grp_s == NG - 1, 2 * CAPS // 16, 4 * CAPS // 16)
    colbase = win_s * idxcols_per_w
    IDXCOLS = 4 * (4 * CAPS // 16)
    idx16 = np.zeros((NCORES, NG, 16, IDXCOLS), dtype=np.int16)
    idx16[core_s, grp_s, slot % 16, colbase + slot // 16] = idx_s

    cc = pos // P
    m = (tloc_s * NW + win_s) * C_tw + cc
    CPG = 4 * NW * C_tw
    meta = np.zeros((NCORES, NG, P, 2 * CPG), dtype=np.float32)
    lane = pos % P
    meta[core_s, grp_s, lane, 2 * m] = rel_s
    meta[core_s, grp_s, lane, 2 * m + 1] = w_s

    idx16_full = np.broadcast_to(
        idx16[:, :, None, :, :], (NCORES, NG, 8, 16, IDXCOLS)
    ).reshape(NCORES, NG, P, IDXCOLS)

    return (np.ascontiguousarray(idx16_full), meta, featT, hXT,
            lhs1, lhs2, C_tw)


def _build_program(C):  # C == C_tw
    import concourse.bacc as bacc
    import concourse.tile as tile
    from concourse import bass, mybir
    from concourse.masks import make_identity

    f32 = mybir.dt.float32
    i16 = mybir.dt.int16
    gdtype = mybir.dt.bfloat16

    nc = bacc.Bacc("TRN2", target_bir_lowering=False, num_swdge_queues=4)

    C_tw = C
    CAPS = C_tw * P
    IDXCOLS = 4 * (4 * CAPS // 16)
    CPG = 4 * NW * C_tw

    featT_p = nc.declare_dram_parameter("featT", [IN + 1, NPC_PAD], f32, isOutput=False)
    hXT_p = nc.declare_dram_parameter("hXT", [IN + 1, NPC_PAD], f32, isOutput=False)
    lhs1_p = nc.declare_dram_parameter("lhs1", [IN + 1, H], f32, isOutput=False)
    lhs2_p = nc.declare_dram_parameter("lhs2", [IN + 1, H], f32, isOutput=False)
    idx_p = nc.declare_dram_parameter("idx", [NG, P, IDXCOLS], i16, isOutput=False)
    meta_p = nc.declare_dram_parameter("meta", [NG, P, 2 * CPG], f32, isOutput=False)
    Wself_p = nc.declare_dram_parameter("Wself", [L, H, H], f32, isOutput=False)
    Wneigh_p = nc.declare_dram_parameter("Wneigh", [L, H, H], f32, isOutput=False)
    bng_p = nc.declare_dram_parameter("bng", [L, H, 1], f32, isOutput=False)
    bnb_p = nc.declare_dram_parameter("bnb", [L, H, 1], f32, isOutput=False)
    W1_p = nc.declare_dram_parameter("W1", [H, H // 2], f32, isOutput=False)
    b1_p = nc.declare_dram_parameter("b1", [H // 2, 1], f32, isOutput=False)
    W2_p = nc.declare_dram_parameter("W2", [H // 2, OUT], f32, isOutput=False)
    b2_p = nc.declare_dram_parameter("b2", [OUT, 1], f32, isOutput=False)
    out_p = nc.declare_dram_parameter("out", [OUT, NPC_PAD], f32, isOutput=True)

    groups = [(s, min(GW, NPC_PAD - s)) for s in range(0, NPC_PAD, GW)]
    rg = [list(range(NCORES))]

    with tile.TileContext(nc) as tc:
        with (
            tc.tile_pool(name="persist", bufs=1) as pp,
            tc.tile_pool(name="pdram", bufs=1, space="DRAM") as pd,
            tc.tile_pool(name="gpool", bufs=8) as gpool,
            tc.tile_pool(name="ohpool", bufs=8) as ohpool,
            tc.tile_pool(name="mpool", bufs=3) as mpool,
            tc.tile_pool(name="grp", bufs=2) as grp,
            tc.tile_pool(name="small", bufs=8) as small,
            tc.tile_pool(name="trp", bufs=4) as trp,
            tc.tile_pool(name="mps", bufs=4, space="PSUM") as mps,
            tc.tile_pool(name="xps", bufs=2, space="PSUM") as xps,
            tc.tile_pool(name="tps", bufs=2, space="PSUM") as tps,
        ):
            # --- persistent SBUF state ---
            bufA = pp.tile([P, NPC_PAD], f32, tag="bufA", name="bufA")
            bufB = pp.tile([P, NPC_PAD], f32, tag="bufB", name="bufB")
            iota_t = pp.tile([P, P], f32, tag="iota_t", name="iota_t")
            nc.gpsimd.iota(iota_t[:], pattern=[[1, P]], base=0,
                           channel_multiplier=0,
                           allow_small_or_imprecise_dtypes=True)
            ident = pp.tile([P, P], f32, tag="ident", name="ident")
            make_identity(nc, ident[:])
            eps_t = pp.tile([P, 1], f32, tag="eps_t", name="eps_t")
            nc.gpsimd.memset(eps_t[:], EPS)

            lhs1_t = pp.tile([IN + 1, H], f32, tag="lhs1_t", name="lhs1_t")
            nc.sync.dma_start(lhs1_t[:], lhs1_p[:])
            lhs2_t = pp.tile([IN + 1, H], f32, tag="lhs2_t", name="lhs2_t")
            nc.sync.dma_start(lhs2_t[:], lhs2_p[:])
            W1_t = pp.tile([H, H // 2], f32, tag="W1_t", name="W1_t")
            nc.sync.dma_start(W1_t[:], W1_p[:])
            b1_t = pp.tile([H // 2, 1], f32, tag="b1_t", name="b1_t")
            nc.sync.dma_start(b1_t[:], b1_p[:])
            W2_t = pp.tile([H // 2, OUT], f32, tag="W2_t", name="W2_t")
            nc.sync.dma_start(W2_t[:], W2_p[:])
            b2_t = pp.tile([OUT, 1], f32, tag="b2_t", name="b2_t")
            nc.sync.dma_start(b2_t[:], b2_p[:])

            # --- internal DRAM: per-quarter AG inputs + window tables ---
            ag_q = [
                [pd.tile([QS[q], H], gdtype, tag=f"ag{l}_{q}", name=f"ag{l}_{q}")
                 for q in range(NW)]
                for l in range(2)
            ]
            tab = [
                [pd.tile([WR[q], H], gdtype, addr_space="Shared",
                         tag=f"tab{l}_{q}", name=f"tab{l}_{q}")
                 for q in range(NW)]
                for l in range(2)
            ]
            ar_in = [
                pd.tile([P, 2], f32, tag=f"ar_in{l}", name=f"ar_in{l}")
                for l in range(L)
            ]
            ar_out = [
                pd.tile([P, 2], f32, addr_space="Shared",
                        tag=f"ar_out{l}", name=f"ar_out{l}")
                for l in range(L)
            ]

            def transpose_to(l):
                """bufA (feature-major) -> node-major quarters -> ag_q[l],
                AllGathered per quarter into tab[l][q] so window-q gathers
                of the next layer start as soon as chunk q lands."""
                kt = 0
                for q in range(NW):
                    nt_q = QT[q]
                    done = 0
                    while done < nt_q:
                        nstage = min(4, nt_q - done)
                        stage = trp.tile([P, GW], gdtype, tag="tr")
                        for k in range(nstage):
                            t = kt + done + k
                            ps = tps.tile([P, P], f32, tag="tps")
                            nc.tensor.transpose(
                                out=ps[:], in_=bufA[:, t * P:(t + 1) * P],
                                identity=ident[:],
                            )
                            nc.scalar.copy(out=stage[:, k * P:(k + 1) * P],
                                           in_=ps[:])
                        so = done * P
                        w = nstage * P
                        nc.scalar.dma_start(
                            ag_q[l][q][so:so + w, :].rearrange(
                                "(b p) f -> p b f", p=P),
                            stage[:, :w].rearrange("p (b f) -> p b f", f=P),
                        )
                        done += nstage
                    kt += nt_q
                    nc.gpsimd.collective_compute(
                        "AllGather",
                        mybir.AluOpType.bypass,
                        ins=[ag_q[l][q].opt()],
                        outs=[tab[l][q].opt()],
                        replica_groups=rg,
                    )

            # ---------------- conv layers ----------------
            for l in range(L):
                if l > 0:
                    Wself_t = small.tile([H, H], f32, tag="ws", bufs=2)
                    nc.sync.dma_start(Wself_t[:], Wself_p[l])
                    Wneigh_t = small.tile([H, H], f32, tag="wn", bufs=2)
                    nc.sync.dma_start(Wneigh_t[:], Wneigh_p[l])
                bng_t = small.tile([H, 1], f32, tag="bng", bufs=2)
                nc.sync.dma_start(bng_t[:], bng_p[l])
                bnb_t = small.tile([H, 1], f32, tag="bnb", bufs=2)
                nc.sync.dma_start(bnb_t[:], bnb_p[l])

                s1p = small.tile([P, 32], f32, tag="s1p", bufs=2)
                s2p = small.tile([P, 32], f32, tag="s2p", bufs=2)

                # ---- phase 1: messages + GEMM + stats ----
                for gi, (s, w) in enumerate(groups):
                    ps = xps.tile([P, GW], f32, tag="xps")
                    if l == 0:
                        fa = mpool.tile([IN + 1, GW], f32, tag="fa")
                        nc.sync.dma_start(fa[:, :w], featT_p[:, s:s + w])
                        fx = mpool.tile([IN + 1, GW], f32, tag="fx")
                        nc.sync.dma_start(fx[:, :w], hXT_p[:, s:s + w])
                        nc.tensor.matmul(out=ps[:, :w], lhsT=lhs1_t[:],
                                         rhs=fa[:, :w], start=True, stop=False)
                        nc.tensor.matmul(out=ps[:, :w], lhsT=lhs2_t[:],
                                         rhs=fx[:, :w], start=False, stop=True)
                    else:
                        ntg = (w + P - 1) // P
                        capw = ntg * CAPS          # idxs per (group, window)
                        idxc_w = capw // 16
                        it = mpool.tile([P, IDXCOLS], i16, tag="idx")
                        nc.sync.dma_start(it[:, :4 * idxc_w],
                                          idx_p[gi][:, :4 * idxc_w])
                        mt = mpool.tile([P, 2 * CPG], f32, tag="meta")
                        nc.scalar.dma_start(mt[:], meta_p[gi])
                        Gs = []
                        for wi in range(NW):
                            gw_t = gpool.tile([P, 4 * CAPS // P * H], gdtype,
                                              tag="g")
                            nc.gpsimd.dma_gather(
                                out_ap=gw_t[:, :capw // P * H].rearrange(
                                    "p (c e) -> p c e", e=H),
                                in_ap=tab[l - 1][wi][:],
                                idxs_ap=it[:, wi * idxc_w:(wi + 1) * idxc_w],
                                num_idxs=capw,
                                num_idxs_reg=capw,
                                elem_size=H,
                                queue_num=wi,
                            )
                            Gs.append(gw_t)
                        msg_ps = mps.tile([P, GW], f32, tag="mps")
                        for t in range(ntg):
                            nmm = NW * C_tw
                            i_mm = 0
                            for wi in range(NW):
                                for cc2 in range(C_tw):
                                    ci = t * C_tw + cc2
                                    mcol = (t * NW + wi) * C_tw + cc2
                                    oh = ohpool.tile([P, P], gdtype, tag="oh")
                                    nc.vector.tensor_scalar(
                                        out=oh[:],
                                        in0=iota_t[:],
                                        scalar1=mt[:, 2 * mcol:2 * mcol + 1],
                                        scalar2=mt[:, 2 * mcol + 1:2 * mcol + 2],
                                        op0=mybir.AluOpType.is_equal,
                                        op1=mybir.AluOpType.mult,
                                    )
                                    nc.tensor.matmul(
                                        out=msg_ps[:, t * P:(t + 1) * P],
                                        lhsT=Gs[wi][:, ci * H:(ci + 1) * H],
                                        rhs=oh[:],
                                        start=(i_mm == 0), stop=(i_mm == nmm - 1),
                                    )
                                    i_mm += 1
                        msg_g = grp.tile([P, GW], f32, tag="msg")
                        nc.scalar.copy(out=msg_g[:, :w], in_=msg_ps[:, :w])

                        nc.tensor.matmul(out=ps[:, :w], lhsT=Wself_t[:],
                                         rhs=bufA[:, s:s + w],
                                         start=True, stop=False)
                        nc.tensor.matmul(out=ps[:, :w], lhsT=Wneigh_t[:],
                                         rhs=msg_g[:, :w],
                                         start=False, stop=True)
                    nc.vector.tensor_copy(out=bufB[:, s:s + w], in_=ps[:, :w])

                    wr = w if s + w <= NPC else max(0, NPC - s)
                    if wr > 0:
                        nc.vector.reduce_sum(
                            s1p[:, gi:gi + 1], bufB[:, s:s + wr],
                            axis=mybir.AxisListType.X,
                        )
                        sq = grp.tile([P, GW], f32, tag="sq")
                        nc.scalar.activation(
                            sq[:, :wr], bufB[:, s:s + wr],
                            mybir.ActivationFunctionType.Square,
                            accum_out=s2p[:, gi:gi + 1],
                        )

                # ---- BN stats all-reduce ----
                st = small.tile([P, 2], f32, tag="st", bufs=2)
                nc.vector.reduce_sum(st[:, 0:1], s1p[:, :len(groups)],
                                     axis=mybir.AxisListType.X)
                nc.vector.reduce_sum(st[:, 1:2], s2p[:, :len(groups)],
                                     axis=mybir.AxisListType.X)
                nc.sync.dma_start(ar_in[l][:], st[:])
                nc.gpsimd.collective_compute(
                    "AllReduce",
                    mybir.AluOpType.add,
                    ins=[ar_in[l].opt()],
                    outs=[ar_out[l].opt()],
                    replica_groups=rg,
                )
                sg = small.tile([P, 2], f32, tag="sg", bufs=2)
                nc.sync.dma_start(sg[:], ar_out[l][:])

                mu = small.tile([P, 1], f32, tag="mu", bufs=2)
                nc.vector.tensor_scalar_mul(mu[:], sg[:, 0:1], 1.0 / N)
                ex2 = small.tile([P, 1], f32, tag="ex2", bufs=2)
                nc.vector.tensor_scalar_mul(ex2[:], sg[:, 1:2], 1.0 / N)
                var = small.tile([P, 1], f32, tag="var", bufs=2)
                nc.vector.tensor_tensor(out=var[:], in0=mu[:], in1=mu[:],
                                        op=mybir.AluOpType.mult)
                nc.vector.tensor_tensor(out=var[:], in0=ex2[:], in1=var[:],
                                        op=mybir.AluOpType.subtract)
                sd = small.tile([P, 1], f32, tag="sd", bufs=2)
                nc.scalar.activation(sd[:], var[:],
                                     mybir.ActivationFunctionType.Sqrt,
                                     bias=eps_t[:])
                rstd = small.tile([P, 1], f32, tag="rstd", bufs=2)
                nc.vector.reciprocal(rstd[:], sd[:])
                a_t = small.tile([P, 1], f32, tag="a_t", bufs=2)
                nc.vector.tensor_tensor(out=a_t[:], in0=bng_t[:], in1=rstd[:],
                                        op=mybir.AluOpType.mult)
                b_t = small.tile([P, 1], f32, tag="b_t", bufs=2)
                nc.vector.tensor_tensor(out=b_t[:], in0=mu[:], in1=a_t[:],
                                        op=mybir.AluOpType.mult)
                nc.vector.tensor_tensor(out=b_t[:], in0=bnb_t[:], in1=b_t[:],
                                        op=mybir.AluOpType.subtract)

                # ---- phase 2: BN + ReLU (+ residual) ----
                for (s, w) in groups:
                    y = grp.tile([P, GW], f32, tag="y")
                    nc.vector.tensor_scalar(
                        out=y[:, :w], in0=bufB[:, s:s + w],
                        scalar1=a_t[:], scalar2=b_t[:],
                        op0=mybir.AluOpType.mult, op1=mybir.AluOpType.add,
                    )
                    if l == 0:
                        nc.scalar.activation(
                            bufA[:, s:s + w], y[:, :w],
                            mybir.ActivationFunctionType.Relu,
                        )
                    else:
                        y2 = grp.tile([P, GW], f32, tag="y2")
                        nc.scalar.activation(
                            y2[:, :w], y[:, :w],
                            mybir.ActivationFunctionType.Relu,
                        )
                        nc.vector.tensor_tensor(
                            out=bufA[:, s:s + w], in0=y2[:, :w],
                            in1=bufA[:, s:s + w], op=mybir.AluOpType.add,
                        )
                nc.gpsimd.memset(bufA[:, NPC:], 0.0)

                if l + 1 < L:
                    transpose_to(l)

            # ---------------- MLP head ----------------
            for (s, w) in groups:
                ps1 = xps.tile([H // 2, GW], f32, tag="xps")
                nc.tensor.matmul(out=ps1[:, :w], lhsT=W1_t[:],
                                 rhs=bufA[:, s:s + w], start=True, stop=True)
                z1 = grp.tile([H // 2, GW], f32, tag="z1")
                nc.scalar.activation(
                    z1[:, :w], ps1[:, :w],
                    mybir.ActivationFunctionType.Relu, bias=b1_t[:],
                )
                ps2 = tps.tile([OUT, GW], f32, tag="tps")
                nc.tensor.matmul(out=ps2[:, :w], lhsT=W2_t[:],
                                 rhs=z1[:, :w], start=True, stop=True)
                o = trp.tile([OUT, GW], f32, tag="tr")
                nc.vector.tensor_scalar_add(o[:, :w], ps2[:, :w], b2_t[:])
                nc.sync.dma_start(out_p[:, s:s + w], o[:, :w])

    nc.compile()
    return nc


def kernel(**inputs):
    from concourse.bass_utils import run_bass_kernel_spmd

    idx, meta, featT, hXT, lhs1, lhs2, C = _host_prep(**inputs)

    key = ("prog", C)
    if key not in _cache:
        _cache[key] = _build_program(C)
    nc = _cache[key]

    f32 = np.float32
    Wself = np.ascontiguousarray(np.asarray(inputs["Wself"], f32))
    Wneigh = np.ascontiguousarray(np.asarray(inputs["Wneigh"], f32))
    bng = np.asarray(inputs["bn_gamma"], f32).reshape(L, H, 1)
    bnb = np.asarray(inputs["bn_beta"], f32).reshape(L, H, 1)
    W1 = np.ascontiguousarray(np.asarray(inputs["W1"], f32))
    b1 = np.asarray(inputs["b1"], f32).reshape(H // 2, 1)
    W2 = np.ascontiguousarray(np.asarray(inputs["W2"], f32))
    b2 = np.asarray(inputs["b2"], f32).reshape(OUT, 1)

    in_maps = []
    for c in range(NCORES):
        in_maps.append({
            "featT": np.ascontiguousarray(featT[c]),
            "hXT": np.ascontiguousarray(hXT[c]),
            "lhs1": np.ascontiguousarray(lhs1),
            "lhs2": np.ascontiguousarray(lhs2),
            "idx": np.ascontiguousarray(idx[c]),
            "meta": np.ascontiguousarray(meta[c]),
            "Wself": Wself, "Wneigh": Wneigh,
            "bng": bng, "bnb": bnb,
            "W1": W1, "b1": b1, "W2": W2, "b2": b2,
        })

    global _last_in_maps
    _last_in_maps = in_maps

    res = run_bass_kernel_spmd(nc, in_maps, list(range(NCORES))).results
    out = np.concatenate(
        [res[c]["out"][:, :NPC].T for c in range(NCORES)], axis=0
    )
    return out.astype(np.float32)


if __name__ == "__main__":
    pass


# revision 10
# speedup vs baseline: 2.4389x; 1.1127x over previous
"""CongestionGCN on 8 Trainium2 NeuronCores.

Graph/data-parallel sharding: nodes split contiguously across 8 cores
(12500 each, padded to 12544 = 98*128). Edges partitioned by dst node.

v2 layout:
- Layer 1's message aggregation is linear in the raw inputs, so the
  host folds it away: msg1 = (S_w X) @ embW + mask*emb_b, and both the
  Wself and Wneigh contributions collapse to two 13-row GEMMs against
  host-shipped [X^T;1] and [(S_w X)^T;mask]. No gather, no AllGather,
  no embedding GEMM for layer 1.
- Layers 2/3 message passing: indirect-DMA gather of src rows from a
  node-major table (4 windows = src quarters across all cores, int16
  addressable), spread across the 4 SWDGE queues so the Q7 pairs
  generate descriptors in parallel. A one-hot matmul on the tensor
  engine performs scatter-add + mean scaling, accumulating msg^T in
  PSUM.
- Each layer's output is transposed back to node-major and AllGathered
  in 4 quarter chunks, so window-q gathers of the next layer start as
  soon as chunk q lands.
- BatchNorm batch stats cross-core via AllReduce. conv_b is dropped:
  BN subtracts the batch mean, so a pre-BN per-feature bias cancels.
"""

import numpy as np

N = 100000
E = 600000
IN = 12
H = 128
OUT = 2
L = 3
EPS = 1e-5

NCORES = 8
P = 128
NPC = N // NCORES            # 12500 real nodes per core
NT = (NPC + P - 1) // P      # 98 dst tiles per core
NPC_PAD = NT * P             # 12544
GW = 512                     # free-dim group width for dense GEMM / BN passes

NW = 4                       # src windows = shard quarters
QB = [0, 3200, 6400, 9472, 12544]      # quarter boundaries (128-aligned)
QS = [3200, 3200, 3072, 3072]          # quarter sizes
WR = [8 * q for q in QS]               # window table rows (<= int16 range)
QT = [25, 25, 24, 24]                  # tiles per quarter
NG = (NT + 3) // 4           # 25 groups of up to 4 dst tiles (512 dsts)

_cache = {}


def _host_prep(features, edge_index, emb_W, emb_b, Wself, Wneigh, **kw):
    src = np.asarray(edge_index[0], dtype=np.int64)
    dst = np.asarray(edge_index[1], dtype=np.int64)
    X = np.asarray(features, dtype=np.float32)

    deg = np.bincount(dst, minlength=N).astype(np.float32)
    inv_deg = (1.0 / np.maximum(deg, 1.0)).astype(np.float32)

    # ---- layer-1 folding: SwX = diag(inv_deg) * segment_sum(X[src], dst)
    SwX = np.empty((N, IN), np.float32)
    for j in range(IN):
        SwX[:, j] = np.bincount(dst, weights=X[src, j], minlength=N)
    SwX *= inv_deg[:, None]
    mask = (deg > 0).astype(np.float32)

    emb_W = np.asarray(emb_W, np.float32)
    emb_b = np.asarray(emb_b, np.float32)
    Ws0 = np.asarray(Wself[0], np.float32)
    Wn0 = np.asarray(Wneigh[0], np.float32)
    lhs1 = np.vstack([emb_W @ Ws0, emb_b[None, :] @ Ws0])   # [13, H]
    lhs2 = np.vstack([emb_W @ Wn0, emb_b[None, :] @ Wn0])   # [13, H]

    featT = np.zeros((NCORES, IN + 1, NPC_PAD), np.float32)
    hXT = np.zeros((NCORES, IN + 1, NPC_PAD), np.float32)
    for c in range(NCORES):
        sl = slice(c * NPC, (c + 1) * NPC)
        featT[c, :IN, :NPC] = X[sl].T
        featT[c, IN, :NPC] = 1.0
        hXT[c, :IN, :NPC] = SwX[sl].T
        hXT[c, IN, :NPC] = mask[sl]

    # ---- edge partitioning for layers 2/3 gathers
    core = dst // NPC
    ltile = (dst % NPC) // P                 # 0..97
    group = ltile // 4                       # 0..24
    tloc = ltile % 4
    dst_rel = (dst % NPC) % P
    src_c = src // NPC
    src_r = src % NPC
    win = np.searchsorted(QB, src_r, side='right') - 1
    qs = np.asarray(QS, np.int64)
    qb = np.asarray(QB[:4], np.int64)
    idx_rel = (src_c * qs[win] + (src_r - qb[win])).astype(np.int16)

    cell = ((core * NG + group) * NW + win) * 4 + tloc
    ncell = NCORES * NG * NW * 4
    order = np.lexsort((src, cell))
    cell_s = cell[order]
    counts = np.bincount(cell_s, minlength=ncell)
    C_tw = int((counts.max() + P - 1) // P)
    CAPS = C_tw * P
    starts = np.zeros(ncell, dtype=np.int64)
    starts[1:] = np.cumsum(counts)[:-1]
    pos = np.arange(E, dtype=np.int64) - starts[cell_s]

    idx_s = idx_rel[order]
    rel_s = dst_rel[order].astype(np.float32)
    w_s = inv_deg[dst[order]]
    core_s = core[order]
    grp_s = group[order]
    win_s = win[order]
    tloc_s = tloc[order]

    slot = tloc_s * CAPS + pos
    idxcols_per_w = np.where(grp_s == NG - 1, 2 * CAPS // 16, 4 * CAPS // 16)
    colbase = win_s * idxcols_per_w
    IDXCOLS = 4 * (4 * CAPS // 16)
    idx16 = np.zeros((NCORES, NG, 16, IDXCOLS), dtype=np.int16)
    idx16[core_s, grp_s, slot % 16, colbase + slot // 16] = idx_s

    # host-precomputed one-hot scatter matrices (layer-invariant):
    # oh[c, g, lane, chunk*P + dst_rel] = inv_deg weight
    import ml_dtypes
    cc = pos // P
    m = (tloc_s * NW + win_s) * C_tw + cc
    CPG = 4 * NW * C_tw
    oh = np.zeros((NCORES, NG, P, CPG * P), dtype=ml_dtypes.bfloat16)
    lane = pos % P
    oh[core_s, grp_s, lane, m * P + dst_rel[order]] = w_s

    idx16_full = np.broadcast_to(
        idx16[:, :, None, :, :], (NCORES, NG, 8, 16, IDXCOLS)
    ).reshape(NCORES, NG, P, IDXCOLS)

    return (np.ascontiguousarray(idx16_full), oh, featT, hXT,
            lhs1, lhs2, C_tw)


def _build_program(C):  # C == C_tw
    import concourse.bacc as bacc
    import concourse.tile as tile
    from concourse import bass, mybir
    from concourse.masks import make_identity

    f32 = mybir.dt.float32
    i16 = mybir.dt.int16
    gdtype = mybir.dt.bfloat16

    nc = bacc.Bacc("TRN2", target_bir_lowering=False, num_swdge_queues=4)

    C_tw = C
    CAPS = C_tw * P
    IDXCOLS = 4 * (4 * CAPS // 16)
    CPG = 4 * NW * C_tw

    featT_p = nc.declare_dram_parameter("featT", [IN + 1, NPC_PAD], f32, isOutput=False)
    hXT_p = nc.declare_dram_parameter("hXT", [IN + 1, NPC_PAD], f32, isOutput=False)
    lhs1_p = nc.declare_dram_parameter("lhs1", [IN + 1, H], f32, isOutput=False)
    lhs2_p = nc.declare_dram_parameter("lhs2", [IN + 1, H], f32, isOutput=False)
    idx_p = nc.declare_dram_parameter("idx", [NG, P, IDXCOLS], i16, isOutput=False)
    oh_p = nc.declare_dram_parameter("oh", [NG, P, CPG * P], gdtype, isOutput=False)
    Wself_p = nc.declare_dram_parameter("Wself", [L, H, H], f32, isOutput=False)
    Wneigh_p = nc.declare_dram_parameter("Wneigh", [L, H, H], f32, isOutput=False)
    bng_p = nc.declare_dram_parameter("bng", [L, H, 1], f32, isOutput=False)
    bnb_p = nc.declare_dram_parameter("bnb", [L, H, 1], f32, isOutput=False)
    W1_p = nc.declare_dram_parameter("W1", [H, H // 2], f32, isOutput=False)
    b1_p = nc.declare_dram_parameter("b1", [H // 2, 1], f32, isOutput=False)
    W2_p = nc.declare_dram_parameter("W2", [H // 2, OUT], f32, isOutput=False)
    b2_p = nc.declare_dram_parameter("b2", [OUT, 1], f32, isOutput=False)
    out_p = nc.declare_dram_parameter("out", [OUT, NPC_PAD], f32, isOutput=True)

    groups = [(s, min(GW, NPC_PAD - s)) for s in range(0, NPC_PAD, GW)]
    rg = [list(range(NCORES))]

    with tile.TileContext(nc) as tc:
        with (
            tc.tile_pool(name="persist", bufs=1) as pp,
            tc.tile_pool(name="pdram", bufs=1, space="DRAM") as pd,
            tc.tile_pool(name="gpool", bufs=8) as gpool,
            tc.tile_pool(name="ohpool", bufs=3) as ohpool,
            tc.tile_pool(name="mpool", bufs=3) as mpool,
            tc.tile_pool(name="grp", bufs=2) as grp,
            tc.tile_pool(name="small", bufs=8) as small,
            tc.tile_pool(name="trp", bufs=4) as trp,
            tc.tile_pool(name="mps", bufs=4, space="PSUM") as mps,
            tc.tile_pool(name="xps", bufs=2, space="PSUM") as xps,
            tc.tile_pool(name="tps", bufs=2, space="PSUM") as tps,
        ):
            # --- persistent SBUF state ---
            bufA = pp.tile([P, NPC_PAD], f32, tag="bufA", name="bufA")
            bufB = pp.tile([P, NPC_PAD], f32, tag="bufB", name="bufB")
            ident = pp.tile([P, P], f32, tag="ident", name="ident")
            make_identity(nc, ident[:])
            eps_t = pp.tile([P, 1], f32, tag="eps_t", name="eps_t")
            nc.gpsimd.memset(eps_t[:], EPS)

            lhs1_t = pp.tile([IN + 1, H], f32, tag="lhs1_t", name="lhs1_t")
            nc.sync.dma_start(lhs1_t[:], lhs1_p[:])
            lhs2_t = pp.tile([IN + 1, H], f32, tag="lhs2_t", name="lhs2_t")
            nc.sync.dma_start(lhs2_t[:], lhs2_p[:])
            W1_t = pp.tile([H, H // 2], f32, tag="W1_t", name="W1_t")
            nc.sync.dma_start(W1_t[:], W1_p[:])
            b1_t = pp.tile([H // 2, 1], f32, tag="b1_t", name="b1_t")
            nc.sync.dma_start(b1_t[:], b1_p[:])
            W2_t = pp.tile([H // 2, OUT], f32, tag="W2_t", name="W2_t")
            nc.sync.dma_start(W2_t[:], W2_p[:])
            b2_t = pp.tile([OUT, 1], f32, tag="b2_t", name="b2_t")
            nc.sync.dma_start(b2_t[:], b2_p[:])

            # --- internal DRAM: per-quarter AG inputs + window tables ---
            ag_q = [
                [pd.tile([QS[q], H], gdtype, tag=f"ag{l}_{q}", name=f"ag{l}_{q}")
                 for q in range(NW)]
                for l in range(2)
            ]
            tab = [
                [pd.tile([WR[q], H], gdtype, addr_space="Shared",
                         tag=f"tab{l}_{q}", name=f"tab{l}_{q}")
                 for q in range(NW)]
                for l in range(2)
            ]
            ar_in = [
                pd.tile([P, 2], f32, tag=f"ar_in{l}", name=f"ar_in{l}")
                for l in range(L)
            ]
            ar_out = [
                pd.tile([P, 2], f32, addr_space="Shared",
                        tag=f"ar_out{l}", name=f"ar_out{l}")
                for l in range(L)
            ]

            def transpose_to(l):
                """bufA (feature-major) -> node-major quarters -> ag_q[l],
                AllGathered per quarter into tab[l][q] so window-q gathers
                of the next layer start as soon as chunk q lands."""
                kt = 0
                for q in range(NW):
                    nt_q = QT[q]
                    done = 0
                    while done < nt_q:
                        nstage = min(4, nt_q - done)
                        stage = trp.tile([P, GW], gdtype, tag="tr")
                        for k in range(nstage):
                            t = kt + done + k
                            ps = tps.tile([P, P], f32, tag="tps")
                            nc.tensor.transpose(
                                out=ps[:], in_=bufA[:, t * P:(t + 1) * P],
                                identity=ident[:],
                            )
                            nc.scalar.copy(out=stage[:, k * P:(k + 1) * P],
                                           in_=ps[:])
                        so = done * P
                        w = nstage * P
                        nc.scalar.dma_start(
                            ag_q[l][q][so:so + w, :].rearrange(
                                "(b p) f -> p b f", p=P),
                            stage[:, :w].rearrange("p (b f) -> p b f", f=P),
                        )
                        done += nstage
                    kt += nt_q
                    nc.gpsimd.collective_compute(
                        "AllGather",
                        mybir.AluOpType.bypass,
                        ins=[ag_q[l][q].opt()],
                        outs=[tab[l][q].opt()],
                        replica_groups=rg,
                    )

            # ---------------- conv layers ----------------
            for l in range(L):
                if l > 0:
                    Wself_t = small.tile([H, H], f32, tag="ws", bufs=2)
                    nc.sync.dma_start(Wself_t[:], Wself_p[l])
                    Wneigh_t = small.tile([H, H], f32, tag="wn", bufs=2)
                    nc.sync.dma_start(Wneigh_t[:], Wneigh_p[l])
                bng_t = small.tile([H, 1], f32, tag="bng", bufs=2)
                nc.sync.dma_start(bng_t[:], bng_p[l])
                bnb_t = small.tile([H, 1], f32, tag="bnb", bufs=2)
                nc.sync.dma_start(bnb_t[:], bnb_p[l])

                s1p = small.tile([P, 32], f32, tag="s1p", bufs=2)
                s2p = small.tile([P, 32], f32, tag="s2p", bufs=2)

                # ---- phase 1: messages + GEMM + stats ----
                for gi, (s, w) in enumerate(groups):
                    ps = xps.tile([P, GW], f32, tag="xps")
                    if l == 0:
                        fa = mpool.tile([IN + 1, GW], f32, tag="fa")
                        nc.sync.dma_start(fa[:, :w], featT_p[:, s:s + w])
                        fx = mpool.tile([IN + 1, GW], f32, tag="fx")
                        nc.sync.dma_start(fx[:, :w], hXT_p[:, s:s + w])
                        nc.tensor.matmul(out=ps[:, :w], lhsT=lhs1_t[:],
                                         rhs=fa[:, :w], start=True, stop=False)
                        nc.tensor.matmul(out=ps[:, :w], lhsT=lhs2_t[:],
                                         rhs=fx[:, :w], start=False, stop=True)
                    else:
                        ntg = (w + P - 1) // P
                        capw = ntg * CAPS          # idxs per (group, window)
                        idxc_w = capw // 16
                        it = mpool.tile([P, IDXCOLS], i16, tag="idx")
                        nc.sync.dma_start(it[:, :4 * idxc_w],
                                          idx_p[gi][:, :4 * idxc_w])
                        oh_t = ohpool.tile([P, CPG * P], gdtype, tag="oh")
                        ncols = ntg * NW * C_tw * P
                        oh_eng = nc.sync if gi % 2 == 0 else nc.scalar
                        oh_eng.dma_start(oh_t[:, :ncols], oh_p[gi][:, :ncols])
                        Gs = []
                        for wi in range(NW):
                            gw_t = gpool.tile([P, 4 * CAPS // P * H], gdtype,
                                              tag="g")
                            nc.gpsimd.dma_gather(
                                out_ap=gw_t[:, :capw // P * H].rearrange(
                                    "p (c e) -> p c e", e=H),
                                in_ap=tab[l - 1][wi][:],
                                idxs_ap=it[:, wi * idxc_w:(wi + 1) * idxc_w],
                                num_idxs=capw,
                                num_idxs_reg=capw,
                                elem_size=H,
                                queue_num=wi,
                            )
                            Gs.append(gw_t)
                        msg_ps = mps.tile([P, GW], f32, tag="mps")
                        for t in range(ntg):
                            nmm = NW * C_tw
                            i_mm = 0
                            for wi in range(NW):
                                for cc2 in range(C_tw):
                                    ci = t * C_tw + cc2
                                    mcol = (t * NW + wi) * C_tw + cc2
                                    nc.tensor.matmul(
                                        out=msg_ps[:, t * P:(t + 1) * P],
                                        lhsT=Gs[wi][:, ci * H:(ci + 1) * H],
                                        rhs=oh_t[:, mcol * P:(mcol + 1) * P],
                                        start=(i_mm == 0), stop=(i_mm == nmm - 1),
                                    )
                                    i_mm += 1
                        msg_g = grp.tile([P, GW], f32, tag="msg")
                        nc.scalar.copy(out=msg_g[:, :w], in_=msg_ps[:, :w])

                        nc.tensor.matmul(out=ps[:, :w], lhsT=Wself_t[:],
                                         rhs=bufA[:, s:s + w],
                                         start=True, stop=False)
                        nc.tensor.matmul(out=ps[:, :w], lhsT=Wneigh_t[:],
                                         rhs=msg_g[:, :w],
                                         start=False, stop=True)
                    nc.vector.tensor_copy(out=bufB[:, s:s + w], in_=ps[:, :w])

                    wr = w if s + w <= NPC else max(0, NPC - s)
                    if wr > 0:
                        nc.vector.reduce_sum(
                            s1p[:, gi:gi + 1], bufB[:, s:s + wr],
                            axis=mybir.AxisListType.X,
                        )
                        sq = grp.tile([P, GW], f32, tag="sq")
                        nc.scalar.activation(
                            sq[:, :wr], bufB[:, s:s + wr],
                            mybir.ActivationFunctionType.Square,
                            accum_out=s2p[:, gi:gi + 1],
                        )

                # ---- BN stats all-reduce ----
                st = small.tile([P, 2], f32, tag="st", bufs=2)
                nc.vector.reduce_sum(st[:, 0:1], s1p[:, :len(groups)],
                                     axis=mybir.AxisListType.X)
                nc.vector.reduce_sum(st[:, 1:2], s2p[:, :len(groups)],
                                     axis=mybir.AxisListType.X)
                nc.sync.dma_start(ar_in[l][:], st[:])
                nc.gpsimd.collective_compute(
                    "AllReduce",
                    mybir.AluOpType.add,
                    ins=[ar_in[l].opt()],
                    outs=[ar_out[l].opt()],
                    replica_groups=rg,
                )
                sg = small.tile([P, 2], f32, tag="sg", bufs=2)
                nc.sync.dma_start(sg[:], ar_out[l][:])

                mu = small.tile([P, 1], f32, tag="mu", bufs=2)
                nc.vector.tensor_scalar_mul(mu[:], sg[:, 0:1], 1.0 / N)
                ex2 = small.tile([P, 1], f32, tag="ex2", bufs=2)
                nc.vector.tensor_scalar_mul(ex2[:], sg[:, 1:2], 1.0 / N)
                var = small.tile([P, 1], f32, tag="var", bufs=2)
                nc.vector.tensor_tensor(out=var[:], in0=mu[:], in1=mu[:],
                                        op=mybir.AluOpType.mult)
                nc.vector.tensor_tensor(out=var[:], in0=ex2[:], in1=var[:],
                                        op=mybir.AluOpType.subtract)
                sd = small.tile([P, 1], f32, tag="sd", bufs=2)
                nc.scalar.activation(sd[:], var[:],
                                     mybir.ActivationFunctionType.Sqrt,
                                     bias=eps_t[:])
                rstd = small.tile([P, 1], f32, tag="rstd", bufs=2)
                nc.vector.reciprocal(rstd[:], sd[:])
                a_t = small.tile([P, 1], f32, tag="a_t", bufs=2)
                nc.vector.tensor_tensor(out=a_t[:], in0=bng_t[:], in1=rstd[:],
                                        op=mybir.AluOpType.mult)
                b_t = small.tile([P, 1], f32, tag="b_t", bufs=2)
                nc.vector.tensor_tensor(out=b_t[:], in0=mu[:], in1=a_t[:],
                                        op=mybir.AluOpType.mult)
                nc.vector.tensor_tensor(out=b_t[:], in0=bnb_t[:], in1=b_t[:],
                                        op=mybir.AluOpType.subtract)

                # ---- phase 2: BN + ReLU (+ residual) ----
                for (s, w) in groups:
                    y = grp.tile([P, GW], f32, tag="y")
                    nc.vector.tensor_scalar(
                        out=y[:, :w], in0=bufB[:, s:s + w],
                        scalar1=a_t[:], scalar2=b_t[:],
                        op0=mybir.AluOpType.mult, op1=mybir.AluOpType.add,
                    )
                    if l == 0:
                        nc.scalar.activation(
                            bufA[:, s:s + w], y[:, :w],
                            mybir.ActivationFunctionType.Relu,
                        )
                    else:
                        y2 = grp.tile([P, GW], f32, tag="y2")
                        nc.scalar.activation(
                            y2[:, :w], y[:, :w],
                            mybir.ActivationFunctionType.Relu,
                        )
                        nc.vector.tensor_tensor(
                            out=bufA[:, s:s + w], in0=y2[:, :w],
                            in1=bufA[:, s:s + w], op=mybir.AluOpType.add,
                        )
                nc.gpsimd.memset(bufA[:, NPC:], 0.0)

                if l + 1 < L:
                    transpose_to(l)

            # ---------------- MLP head ----------------
            for (s, w) in groups:
                ps1 = xps.tile([H // 2, GW], f32, tag="xps")
                nc.tensor.matmul(out=ps1[:, :w], lhsT=W1_t[:],
                                 rhs=bufA[:, s:s + w], start=True, stop=True)
                z1 = grp.tile([H // 2, GW], f32, tag="z1")
                nc.scalar.activation(
                    z1[:, :w], ps1[:, :w],
                    mybir.ActivationFunctionType.Relu, bias=b1_t[:],
                )
                ps2 = tps.tile([OUT, GW], f32, tag="tps")
                nc.tensor.matmul(out=ps2[:, :w], lhsT=W2_t[:],
                                 rhs=z1[:, :w], start=True, stop=True)
                o = trp.tile([OUT, GW], f32, tag="tr")
                nc.vector.tensor_scalar_add(o[:, :w], ps2[:, :w], b2_t[:])
                nc.sync.dma_start(out_p[:, s:s + w], o[:, :w])

    nc.compile()
    return nc


def kernel(**inputs):
    from concourse.bass_utils import run_bass_kernel_spmd

    idx, oh, featT, hXT, lhs1, lhs2, C = _host_prep(**inputs)

    key = ("prog", C)
    if key not in _cache:
        _cache[key] = _build_program(C)
    nc = _cache[key]

    f32 = np.float32
    Wself = np.ascontiguousarray(np.asarray(inputs["Wself"], f32))
    Wneigh = np.ascontiguousarray(np.asarray(inputs["Wneigh"], f32))
    bng = np.asarray(inputs["bn_gamma"], f32).reshape(L, H, 1)
    bnb = np.asarray(inputs["bn_beta"], f32).reshape(L, H, 1)
    W1 = np.ascontiguousarray(np.asarray(inputs["W1"], f32))
    b1 = np.asarray(inputs["b1"], f32).reshape(H // 2, 1)
    W2 = np.ascontiguousarray(np.asarray(inputs["W2"], f32))
    b2 = np.asarray(inputs["b2"], f32).reshape(OUT, 1)

    in_maps = []
    for c in range(NCORES):
        in_maps.append({
            "featT": np.ascontiguousarray(featT[c]),
            "hXT": np.ascontiguousarray(hXT[c]),
            "lhs1": np.ascontiguousarray(lhs1),
            "lhs2": np.ascontiguousarray(lhs2),
            "idx": np.ascontiguousarray(idx[c]),
            "oh": np.ascontiguousarray(oh[c]),
            "Wself": Wself, "Wneigh": Wneigh,
            "bng": bng, "bnb": bnb,
            "W1": W1, "b1": b1, "W2": W2, "b2": b2,
        })

    global _last_in_maps
    _last_in_maps = in_maps

    res = run_bass_kernel_spmd(nc, in_maps, list(range(NCORES))).results
    out = np.concatenate(
        [res[c]["out"][:, :NPC].T for c in range(NCORES)], axis=0
    )
    return out.astype(np.float32)


if __name__ == "__main__":
    pass
